# revision 11
# baseline (speedup 1.0000x reference)
"""DecoderRNN (embedding -> GRU -> vocab projection -> log_softmax) on 8 trn2 cores.

Sharding: vocab-parallel. Every core runs the (small, latency-bound) B=32 GRU
recurrence replicated; the [B,S,V] logits / log-softmax work — which dominates
memory traffic — is split column-wise over V (4000 vocab rows per core).
Softmax normalizers are combined with one small AllReduce per 4-step token
group, overlapped with the recurrence.

Layouts (per core):
  - All GRU state is kept transposed: h^T as [128 (h%128), kc(4) x b(32)] so
    gate elementwise work runs on 128 partitions with tiny free dims.
  - hp^T = (W_hh h)^T computed as 48 bf16 matmuls (stationary = W_hh^T tile,
    moving = h^T columns) accumulating into one PSUM bank [128, 384].
  - x-projection xp^T precomputed for all S*B tokens in the prologue
    (embedding rows gathered by indirect DMA, transposed on the PE).
  - Logits per 4-step group g (128 tokens): out[tok, v] = 4 bf16 matmuls +
    a K=1 ones-row matmul folding b_out; exp+row-sum fused in one ScalarE
    activation; per-group AllReduce of the [128] partial sums; final
    logp = Ln(exp * (1/S_global)) on ScalarE, DMA'd straight to the output.
"""

import os
import sys

import numpy as np

import concourse.bass as bass
import concourse.tile as tile
from concourse import bacc, mybir, bass_utils
from concourse.masks import make_identity

try:
    import ml_dtypes

    _BF16 = ml_dtypes.bfloat16
except ImportError:  # pragma: no cover
    _BF16 = None

P = 128
B = 32
H = 512
KC = H // P  # 4 h-chunks
G3 = 3 * H  # 1536 gate rows
M12 = G3 // P  # 12 gate m-tiles
N_CORES = 8
V_FULL = 32000
S_FULL = 64

F32 = mybir.dt.float32
BF = mybir.dt.bfloat16
I32 = mybir.dt.int32

LAST_RESULTS = None  # test harness reads exec_time_ns from here


def _nv_chunk(vc):
    # largest divisor of vc that is <= 512 (PSUM bank budget per matmul)
    for nv in (500, 512, 400, 256, 250, 128, 200, 125, 64, 100, 32):
        if vc % nv == 0:
            return nv
    return vc


def build_nc(n_cores=N_CORES, s_len=S_FULL, v_total=V_FULL):
    vc = v_total // n_cores
    nv = _nv_chunk(vc)
    nch = vc // nv
    n_tok = s_len * B
    ng = s_len // 4  # 4-step token groups of 128 tokens
    assert s_len % 4 == 0 and n_tok % P == 0

    nc = bacc.Bacc("TRN2", target_bir_lowering=False, debug=False,
                   num_devices=n_cores)

    # ---- DRAM I/O ----
    idx_d = nc.dram_tensor("idx", [n_tok], I32, kind="ExternalInput").ap()
    emb_d = nc.dram_tensor("emb", [v_total, H], F32, kind="ExternalInput").ap()
    thT_d = nc.dram_tensor("thT", [H, B], F32, kind="ExternalInput").ap()
    wixT_d = nc.dram_tensor("wixT", [H, G3], BF, kind="ExternalInput").ap()
    witT_d = nc.dram_tensor("witT", [H, G3], BF, kind="ExternalInput").ap()
    whhT_d = nc.dram_tensor("whhT", [H, G3], BF, kind="ExternalInput").ap()
    biasx_d = nc.dram_tensor("biasx", [G3], BF, kind="ExternalInput").ap()
    bhhn_d = nc.dram_tensor("bhhn", [P, KC], F32, kind="ExternalInput").ap()
    woutT_d = nc.dram_tensor("woutT", [H, vc], BF, kind="ExternalInput").ap()
    bout_d = nc.dram_tensor("bout", [vc], BF, kind="ExternalInput").ap()

    logp_d = nc.dram_tensor("logp", [B, s_len, vc], F32,
                            kind="ExternalOutput").ap()
    # transposed [H, B]; host flips it back
    hid_d = nc.dram_tensor("hid", [H, B], F32, kind="ExternalOutput").ap()

    with tile.TileContext(nc) as tc:
        with (
            tc.tile_pool(name="persist", bufs=1) as pp,
            tc.tile_pool(name="work", bufs=3) as wp,
            tc.tile_pool(name="hstate", bufs=2) as hp_pool,
            tc.tile_pool(name="expp", bufs=3) as ep,
            tc.tile_pool(name="outp", bufs=4) as op,
            tc.tile_pool(name="pmm", bufs=2, space="PSUM") as pmm,
            tc.tile_pool(name="ptr", bufs=2, space="PSUM") as ptr,
            tc.tile_pool(name="plog", bufs=2, space="PSUM") as plog,
            tc.tile_pool(name="dram", bufs=2 * ng + 2, space="DRAM") as dp,
        ):
            # ---- constants / weights into SBUF ----
            ident = pp.tile([P, P], F32, tag="ident")
            make_identity(nc, ident[:])
            ones_bf = pp.tile([1, P], BF, tag="ones")
            nc.vector.memset(ones_bf[:], 1.0)

            whh_sb = pp.tile([P, KC * M12 * P], BF, tag="whh")
            wix_sb = pp.tile([P, KC * M12 * P], BF, tag="wix")
            wit_sb = pp.tile([P, KC * M12 * P], BF, tag="wit")
            for kc in range(KC):
                for m in range(M12):
                    dst = slice((kc * M12 + m) * P, (kc * M12 + m + 1) * P)
                    src = (slice(kc * P, (kc + 1) * P), slice(m * P, (m + 1) * P))
                    nc.sync.dma_start(whh_sb[:, dst], whhT_d[src])
                    nc.sync.dma_start(wix_sb[:, dst], wixT_d[src])
                    nc.sync.dma_start(wit_sb[:, dst], witT_d[src])

            wout_sb = pp.tile([P, KC * vc], BF, tag="wout")
            for kc in range(KC):
                nc.sync.dma_start(wout_sb[:, kc * vc:(kc + 1) * vc],
                                  woutT_d[kc * P:(kc + 1) * P, :])
            biasx_sb = pp.tile([1, G3], BF, tag="biasx")
            nc.sync.dma_start(biasx_sb[:, :], biasx_d[None, :])
            bout_sb = pp.tile([1, vc], BF, tag="bout")
            nc.sync.dma_start(bout_sb[:, :], bout_d[None, :])
            bhhn_sb = pp.tile([P, KC], F32, tag="bhhn")
            nc.sync.dma_start(bhhn_sb[:], bhhn_d[:])

            thT_sb = pp.tile([P, KC * B], F32, tag="thT")
            for kc in range(KC):
                nc.sync.dma_start(thT_sb[:, kc * B:(kc + 1) * B],
                                  thT_d[kc * P:(kc + 1) * P, :])
            thTr = pp.tile([P, KC * B], BF, tag="thTr")
            nc.scalar.activation(thTr[:], thT_sb[:],
                                 mybir.ActivationFunctionType.Relu)

            # ---- tpT: thought projection + biases [128, 12*32] ----
            pt = pmm.tile([P, M12 * B], F32, tag="hp", space="PSUM")
            for m in range(M12):
                o = pt[:, m * B:(m + 1) * B]
                for kc in range(KC):
                    nc.tensor.matmul(
                        o, wit_sb[:, (kc * M12 + m) * P:(kc * M12 + m + 1) * P],
                        thTr[:, kc * B:(kc + 1) * B],
                        start=(kc == 0), stop=False)
                nc.tensor.matmul(o, biasx_sb[:1, m * P:(m + 1) * P],
                                 ones_bf[:1, :B], start=False, stop=True)
            tpT = pp.tile([P, M12 * B], F32, tag="tpT")
            nc.vector.tensor_copy(tpT[:], pt[:])

            # ---- gather embeddings, transpose, project: xpT ----
            xT = pp.tile([P, KC * n_tok], BF, tag="xT")
            xTv = xT[:].rearrange("p (kc t) -> p kc t", kc=KC)
            for tk in range(n_tok // P):
                idx_sb = wp.tile([P, 1], I32, tag="idx")
                nc.sync.dma_start(idx_sb[:, 0], idx_d[tk * P:(tk + 1) * P])
                xg = wp.tile([P, H], F32, tag="xg")
                nc.gpsimd.indirect_dma_start(
                    out=xg[:], out_offset=None, in_=emb_d[:],
                    in_offset=bass.IndirectOffsetOnAxis(ap=idx_sb[:, :1], axis=0))
                xgr = wp.tile([P, H], F32, tag="xgr")
                nc.scalar.activation(xgr[:], xg[:],
                                     mybir.ActivationFunctionType.Relu)
                for kc in range(KC):
                    tps = ptr.tile([P, P], F32, tag="tps", space="PSUM")
                    nc.tensor.transpose(tps[:], xgr[:, kc * P:(kc + 1) * P],
                                        ident[:])
                    nc.vector.tensor_copy(
                        xTv[:, kc, tk * P:(tk + 1) * P], tps[:])

            xpT = pp.tile([P, M12 * n_tok], BF, tag="xpT")
            xpv = xpT[:].rearrange("p (m t) -> p m t", m=M12)
            tch_sz = min(512, n_tok)
            for m in range(M12):
                tp_b = tpT[:, m * B:(m + 1) * B] \
                    .to_broadcast([P, B, tch_sz // B]) \
                    .rearrange("p b s -> p s b")
                for tch in range(n_tok // tch_sz):
                    px = plog.tile([P, tch_sz], F32, tag="pl", space="PSUM")
                    for kc in range(KC):
                        nc.tensor.matmul(
                            px[:],
                            wix_sb[:, (kc * M12 + m) * P:(kc * M12 + m + 1) * P],
                            xTv[:, kc, tch * tch_sz:(tch + 1) * tch_sz],
                            start=(kc == 0), stop=(kc == KC - 1))
                    nc.vector.tensor_tensor(
                        out=xpv[:, m, tch * tch_sz:(tch + 1) * tch_sz]
                            .rearrange("p (s b) -> p s b", b=B),
                        in0=px[:].rearrange("p (s b) -> p s b", b=B),
                        in1=tp_b,
                        op=mybir.AluOpType.add)

            # ---- GRU recurrence + per-group logits ----
            h32 = hp_pool.tile([P, KC * B], F32, tag="h32")
            nc.vector.memset(h32[:], 0.0)
            hring_prev = hp_pool.tile([P, 4 * KC * B], BF, tag="hring", bufs=3)
            nc.vector.memset(hring_prev[:], 0.0)  # step -1 == zeros
            hring = hring_prev

            for t in range(s_len):
                g, s_l = divmod(t, 4)
                if s_l == 0:
                    hring = hp_pool.tile([P, 4 * KC * B], BF, tag="hring",
                                         bufs=3)
                # ring layout: [128, kc(4) x s(4) x b(32)]
                if s_l == 0:
                    hcol = [hring_prev[:, (kc * 4 + 3) * B:(kc * 4 + 4) * B]
                            for kc in range(KC)]
                else:
                    hcol = [hring[:, (kc * 4 + s_l - 1) * B:
                                  (kc * 4 + s_l) * B] for kc in range(KC)]

                hpd = pmm.tile([P, M12 * B], F32, tag="hp", space="PSUM")
                for m in range(M12):
                    o = hpd[:, m * B:(m + 1) * B]
                    for kc in range(KC):
                        nc.tensor.matmul(
                            o,
                            whh_sb[:, (kc * M12 + m) * P:(kc * M12 + m + 1) * P],
                            hcol[kc],
                            start=(kc == 0), stop=(kc == KC - 1))

                pre_rz = wp.tile([P, 2 * KC * B], F32, tag="pre_rz")
                nc.vector.tensor_tensor(
                    out=pre_rz[:].rearrange("p (m b) -> p m b", b=B),
                    in0=hpd[:, 0:2 * KC * B].rearrange("p (m b) -> p m b", b=B),
                    in1=xpv[:, 0:2 * KC, t * B:(t + 1) * B],
                    op=mybir.AluOpType.add)
                rz = wp.tile([P, 2 * KC * B], F32, tag="rz")
                nc.scalar.activation(rz[:], pre_rz[:],
                                     mybir.ActivationFunctionType.Sigmoid)

                pre_n = wp.tile([P, KC * B], F32, tag="pre_n")
                for c in range(KC):
                    nc.vector.scalar_tensor_tensor(
                        out=pre_n[:, c * B:(c + 1) * B],
                        in0=hpd[:, (2 * KC + c) * B:(2 * KC + c + 1) * B],
                        scalar=bhhn_sb[:, c:c + 1],
                        in1=rz[:, c * B:(c + 1) * B],
                        op0=mybir.AluOpType.add,
                        op1=mybir.AluOpType.mult)
                nc.vector.tensor_tensor(
                    out=pre_n[:].rearrange("p (m b) -> p m b", b=B),
                    in0=pre_n[:].rearrange("p (m b) -> p m b", b=B),
                    in1=xpv[:, 2 * KC:3 * KC, t * B:(t + 1) * B],
                    op=mybir.AluOpType.add)
                nt = wp.tile([P, KC * B], F32, tag="nt")
                nc.scalar.activation(nt[:], pre_n[:],
                                     mybir.ActivationFunctionType.Tanh)

                d = wp.tile([P, KC * B], F32, tag="d")
                nc.vector.tensor_tensor(out=d[:], in0=h32[:], in1=nt[:],
                                        op=mybir.AluOpType.subtract)
                t2 = wp.tile([P, KC * B], F32, tag="t2")
                nc.vector.tensor_tensor(out=t2[:], in0=d[:],
                                        in1=rz[:, KC * B:2 * KC * B],
                                        op=mybir.AluOpType.mult)
                h32 = hp_pool.tile([P, KC * B], F32, tag="h32")
                nc.vector.tensor_tensor(out=h32[:], in0=t2[:], in1=nt[:],
                                        op=mybir.AluOpType.add)
                nc.vector.tensor_copy(
                    hring[:].rearrange("p (kc s b) -> p kc s b",
                                       kc=KC, s=4)[:, :, s_l, :],
                    h32[:].rearrange("p (kc b) -> p kc b", kc=KC))

                # ---- logits for this 4-step group ----
                if s_l == 3:
                    sacc = wp.tile([P, nch], F32, tag="sacc")
                    expt = ep.tile([P, vc], BF, tag="expg")
                    for ch in range(nch):
                        pl = plog.tile([P, nv], F32, tag="pl", space="PSUM")
                        for kc in range(KC):
                            nc.tensor.matmul(
                                pl[:], hring[:, kc * 4 * B:(kc + 1) * 4 * B],
                                wout_sb[:, kc * vc + ch * nv:
                                        kc * vc + (ch + 1) * nv],
                                start=(kc == 0), stop=False)
                        nc.tensor.matmul(pl[:], ones_bf[:1, :P],
                                         bout_sb[:1, ch * nv:(ch + 1) * nv],
                                         start=False, stop=True)
                        nc.scalar.activation(
                            expt[:, ch * nv:(ch + 1) * nv], pl[:],
                            mybir.ActivationFunctionType.Exp,
                            accum_out=sacc[:, ch:ch + 1])
                    sred = wp.tile([P, 1], F32, tag="sred")
                    if nch > 1:
                        nc.vector.tensor_reduce(
                            sred[:], sacc[:], axis=mybir.AxisListType.X,
                            op=mybir.AluOpType.add)
                    else:
                        nc.vector.tensor_copy(sred[:], sacc[:])
                    s_in = dp.tile([P, 1], F32, tag="arin")
                    s_out = dp.tile([P, 1], F32, tag="arout")
                    nc.gpsimd.dma_start(s_in[:], sred[:])
                    nc.gpsimd.collective_compute(
                        "AllReduce", mybir.AluOpType.add,
                        replica_groups=[list(range(n_cores))],
                        ins=[s_in.opt()], outs=[s_out.opt()])
                    sg = wp.tile([P, 1], F32, tag="sg")
                    nc.gpsimd.dma_start(sg[:], s_out[:])
                    rs = wp.tile([P, 1], F32, tag="rs")
                    nc.vector.reciprocal(rs[:], sg[:])
                    for ch in range(nch):
                        lp_t = op.tile([P, nv], F32, tag="lp")
                        nc.scalar.activation(
                            lp_t[:], expt[:, ch * nv:(ch + 1) * nv],
                            mybir.ActivationFunctionType.Ln,
                            scale=rs[:, :1])
                        dst = logp_d[:, 4 * g:4 * g + 4,
                                     ch * nv:(ch + 1) * nv] \
                            .rearrange("b s v -> s b v")
                        nc.sync.dma_start(dst, lp_t[:])
                    hring_prev = hring

            # ---- final hidden state ----
            nc.sync.dma_start(
                hid_d.rearrange("(kc j) b -> j kc b", kc=KC),
                h32[:].rearrange("p (kc b) -> p kc b", kc=KC))

    nc.compile()
    return nc


# ------------------------------------------------------------------
# host side
# ------------------------------------------------------------------

_NC_CACHE = {}


def _get_nc(n_cores, s_len, v_total):
    key = (n_cores, s_len, v_total)
    if key not in _NC_CACHE:
        _NC_CACHE[key] = build_nc(n_cores, s_len, v_total)
    return _NC_CACHE[key]


def make_in_maps(target_seqs, thought, emb, W_ih, W_hh, b_ih, b_hh, W_out,
                 b_out, n_cores=N_CORES):
    target_seqs = np.asarray(target_seqs)
    thought = np.asarray(thought, np.float32)
    emb = np.ascontiguousarray(np.asarray(emb, np.float32))
    W_ih = np.asarray(W_ih, np.float32)
    W_hh = np.asarray(W_hh, np.float32)
    b_ih = np.asarray(b_ih, np.float32)
    b_hh = np.asarray(b_hh, np.float32)
    W_out = np.asarray(W_out, np.float32)
    b_out = np.asarray(b_out, np.float32)

    b_sz, s_len = target_seqs.shape
    v_total = W_out.shape[0]
    vc = v_total // n_cores

    idx = np.ascontiguousarray(
        target_seqs.T.reshape(-1).astype(np.int32))          # tok = s*B + b
    thT = np.ascontiguousarray(thought[0].T)                 # [H, B]
    wixT = np.ascontiguousarray(W_ih[:, :H].T).astype(_BF16)
    witT = np.ascontiguousarray(W_ih[:, H:].T).astype(_BF16)
    whhT = np.ascontiguousarray(W_hh.T).astype(_BF16)
    biasx = (b_ih + np.concatenate([b_hh[:2 * H],
                                    np.zeros(H, np.float32)])).astype(_BF16)
    bhhn = np.ascontiguousarray(b_hh[2 * H:].reshape(KC, P).T)  # [128, KC]
    woutT = np.ascontiguousarray(W_out.T.astype(_BF16))      # [H, V]
    bout = b_out.astype(_BF16)

    shared = dict(idx=idx, emb=emb, thT=thT, wixT=wixT, witT=witT,
                  whhT=whhT, biasx=biasx, bhhn=bhhn)
    in_maps = []
    for c in range(n_cores):
        m = dict(shared)
        m["woutT"] = np.ascontiguousarray(woutT[:, c * vc:(c + 1) * vc])
        m["bout"] = np.ascontiguousarray(bout[c * vc:(c + 1) * vc])
        in_maps.append(m)
    return in_maps


def kernel(target_seqs, thought, emb, W_ih, W_hh, b_ih, b_hh, W_out, b_out):
    global LAST_RESULTS
    target_seqs = np.asarray(target_seqs)
    b_sz, s_len = target_seqs.shape
    v_total = np.asarray(W_out).shape[0]
    n_cores = N_CORES
    vc = v_total // n_cores

    nc = _get_nc(n_cores, s_len, v_total)
    in_maps = make_in_maps(target_seqs, thought, emb, W_ih, W_hh, b_ih, b_hh,
                           W_out, b_out, n_cores)
    res = bass_utils.run_bass_kernel_spmd(
        nc, in_maps, core_ids=list(range(n_cores)),
        trace=bool(os.environ.get("KERNEL_TRACE")))
    LAST_RESULTS = res

    logp = np.concatenate([res.results[c]["logp"] for c in range(n_cores)],
                          axis=2)
    hid = np.ascontiguousarray(res.results[0]["hid"].T)[None]
    return logp, hid


# revision 21
# speedup vs baseline: 1.0583x; 1.0583x over previous
"""DecoderRNN (embedding -> GRU -> vocab projection -> log_softmax) on 8 trn2 cores.

Sharding: vocab-parallel. Every core runs the (small, latency-bound) B=32 GRU
recurrence replicated; the [B,S,V] logits / log-softmax work — which dominates
memory traffic — is split column-wise over V (4000 vocab rows per core).
Softmax normalizers are combined with one small AllReduce per 4-step token
group, overlapped with the recurrence.

Layouts (per core):
  - All GRU state is kept transposed: h^T as [128 (h%128), kc(4) x b(32)] so
    gate elementwise work runs on 128 partitions with tiny free dims.
  - hp^T = (W_hh h)^T computed as 48 bf16 matmuls (stationary = W_hh^T tile,
    moving = h^T columns) accumulating into one PSUM bank [128, 384].
  - x-projection xp^T precomputed for all S*B tokens in the prologue
    (embedding rows gathered by indirect DMA, transposed on the PE).
  - Logits per 4-step group g (128 tokens): out[tok, v] = 4 bf16 matmuls +
    a K=1 ones-row matmul folding b_out; exp+row-sum fused in one ScalarE
    activation; per-group AllReduce of the [128] partial sums; final
    logp = Ln(exp * (1/S_global)) on ScalarE, DMA'd straight to the output.
"""

import os
import sys

import numpy as np

import concourse.bass as bass
import concourse.tile as tile
from concourse import bacc, mybir, bass_utils
from concourse.masks import make_identity

try:
    import ml_dtypes

    _BF16 = ml_dtypes.bfloat16
except ImportError:  # pragma: no cover
    _BF16 = None

P = 128
B = 32
H = 512
KC = H // P  # 4 h-chunks
G3 = 3 * H  # 1536 gate rows
M12 = G3 // P  # 12 gate m-tiles
N_CORES = 8
V_FULL = 32000
S_FULL = 64

F32 = mybir.dt.float32
BF = mybir.dt.bfloat16
I32 = mybir.dt.int32

LAST_RESULTS = None  # test harness reads exec_time_ns from here


def _nv_chunk(vc):
    # largest divisor of vc that is <= 512 (PSUM bank budget per matmul)
    for nv in (500, 512, 400, 256, 250, 128, 200, 125, 64, 100, 32):
        if vc % nv == 0:
            return nv
    return vc


def build_nc(n_cores=N_CORES, s_len=S_FULL, v_total=V_FULL):
    vc = v_total // n_cores
    nv = _nv_chunk(vc)
    nch = vc // nv
    n_tok = s_len * B
    ng = s_len // 4  # 4-step token groups of 128 tokens
    assert s_len % 4 == 0 and n_tok % P == 0

    nc = bacc.Bacc("TRN2", target_bir_lowering=False, debug=False,
                   num_devices=n_cores)

    # ---- DRAM I/O ----
    idx_d = nc.dram_tensor("idx", [n_tok], I32, kind="ExternalInput").ap()
    emb_d = nc.dram_tensor("emb", [v_total, H], F32, kind="ExternalInput").ap()
    thT_d = nc.dram_tensor("thT", [H, B], F32, kind="ExternalInput").ap()
    wixT_d = nc.dram_tensor("wixT", [H, G3], BF, kind="ExternalInput").ap()
    witT_d = nc.dram_tensor("witT", [H, G3], BF, kind="ExternalInput").ap()
    whhT_d = nc.dram_tensor("whhT", [H, G3], BF, kind="ExternalInput").ap()
    biasx_d = nc.dram_tensor("biasx", [G3], BF, kind="ExternalInput").ap()
    bhnr_d = nc.dram_tensor("bhnr", [H], BF, kind="ExternalInput").ap()
    woutT_d = nc.dram_tensor("woutT", [H, vc], BF, kind="ExternalInput").ap()
    bout_d = nc.dram_tensor("bout", [vc], BF, kind="ExternalInput").ap()

    logp_d = nc.dram_tensor("logp", [B, s_len, vc], F32,
                            kind="ExternalOutput").ap()
    # transposed [H, B]; host flips it back
    hid_d = nc.dram_tensor("hid", [H, B], F32, kind="ExternalOutput").ap()

    with tile.TileContext(nc) as tc:
        with (
            tc.tile_pool(name="persist", bufs=1) as pp,
            tc.tile_pool(name="work", bufs=3) as wp,
            tc.tile_pool(name="hstate", bufs=2) as hp_pool,
            tc.tile_pool(name="expp", bufs=3) as ep,
            tc.tile_pool(name="outp", bufs=4) as op,
            tc.tile_pool(name="pmm", bufs=2, space="PSUM") as pmm,
            tc.tile_pool(name="ptr", bufs=2, space="PSUM") as ptr,
            tc.tile_pool(name="plog", bufs=2, space="PSUM") as plog,
            tc.tile_pool(name="dram", bufs=2 * ng + 2, space="DRAM") as dp,
        ):
            # ---- constants / weights into SBUF ----
            ident = pp.tile([P, P], F32, tag="ident")
            make_identity(nc, ident[:])
            ones_bf = pp.tile([1, P], BF, tag="ones")
            nc.vector.memset(ones_bf[:], 1.0)

            whh_sb = pp.tile([P, KC * M12 * P], BF, tag="whh")
            wix_sb = pp.tile([P, KC * M12 * P], BF, tag="wix")
            for kc in range(KC):
                for m in range(M12):
                    dst = slice((kc * M12 + m) * P, (kc * M12 + m + 1) * P)
                    src = (slice(kc * P, (kc + 1) * P), slice(m * P, (m + 1) * P))
                    nc.sync.dma_start(whh_sb[:, dst], whhT_d[src])
                    nc.sync.dma_start(wix_sb[:, dst], wixT_d[src])

            wout_sb = pp.tile([P, KC * vc], BF, tag="wout")
            for kc in range(KC):
                nc.sync.dma_start(wout_sb[:, kc * vc:(kc + 1) * vc],
                                  woutT_d[kc * P:(kc + 1) * P, :])
            biasx_sb = pp.tile([1, G3], BF, tag="biasx")
            nc.sync.dma_start(biasx_sb[:, :], biasx_d[None, :])
            bout_sb = pp.tile([1, vc], BF, tag="bout")
            nc.sync.dma_start(bout_sb[:, :], bout_d[None, :])
            bhnr_sb = pp.tile([1, H], BF, tag="bhnr")
            nc.sync.dma_start(bhnr_sb[:, :], bhnr_d[None, :])

            thT_sb = pp.tile([P, KC * B], F32, tag="thT")
            for kc in range(KC):
                nc.sync.dma_start(thT_sb[:, kc * B:(kc + 1) * B],
                                  thT_d[kc * P:(kc + 1) * P, :])
            thTr = pp.tile([P, KC * B], BF, tag="thTr")
            nc.scalar.activation(thTr[:], thT_sb[:],
                                 mybir.ActivationFunctionType.Relu)

            # ---- tpT: thought projection + biases [128, 12*32] ----
            pt = pmm.tile([P, M12 * B], F32, tag="hp", space="PSUM")
            for m in range(M12):
                o = pt[:, m * B:(m + 1) * B]
                for kc in range(KC):
                    witt = wp.tile([P, P], BF, tag="witt", name="witt")
                    nc.sync.dma_start(
                        witt[:], witT_d[kc * P:(kc + 1) * P, m * P:(m + 1) * P])
                    nc.tensor.matmul(
                        o, witt[:],
                        thTr[:, kc * B:(kc + 1) * B],
                        start=(kc == 0), stop=False)
                nc.tensor.matmul(o, biasx_sb[:1, m * P:(m + 1) * P],
                                 ones_bf[:1, :B], start=False, stop=True)
            tpT = pp.tile([P, M12 * B], F32, tag="tpT")
            nc.vector.tensor_copy(tpT[:], pt[:])

            # ---- gather embeddings, transpose, project: xpT ----
            xT = pp.tile([P, KC * n_tok], BF, tag="xT")
            xTv = xT[:].rearrange("p (kc t) -> p kc t", kc=KC)
            for tk in range(n_tok // P):
                idx_sb = wp.tile([P, 1], I32, tag="idx")
                nc.sync.dma_start(idx_sb[:, 0], idx_d[tk * P:(tk + 1) * P])
                xg = wp.tile([P, H], F32, tag="xg")
                nc.gpsimd.indirect_dma_start(
                    out=xg[:], out_offset=None, in_=emb_d[:],
                    in_offset=bass.IndirectOffsetOnAxis(ap=idx_sb[:, :1], axis=0))
                xgr = wp.tile([P, H], F32, tag="xgr")
                nc.scalar.activation(xgr[:], xg[:],
                                     mybir.ActivationFunctionType.Relu)
                for kc in range(KC):
                    tps = ptr.tile([P, P], F32, tag="tps", space="PSUM")
                    nc.tensor.transpose(tps[:], xgr[:, kc * P:(kc + 1) * P],
                                        ident[:])
                    nc.vector.tensor_copy(
                        xTv[:, kc, tk * P:(tk + 1) * P], tps[:])

            xpT = pp.tile([P, M12 * n_tok], BF, tag="xpT")
            xpv = xpT[:].rearrange("p (m t) -> p m t", m=M12)
            tch_sz = min(512, n_tok)
            for m in range(M12):
                tp_b = tpT[:, m * B:(m + 1) * B] \
                    .to_broadcast([P, B, tch_sz // B]) \
                    .rearrange("p b s -> p s b")
                for tch in range(n_tok // tch_sz):
                    px = plog.tile([P, tch_sz], F32, tag="pl", space="PSUM")
                    for kc in range(KC):
                        nc.tensor.matmul(
                            px[:],
                            wix_sb[:, (kc * M12 + m) * P:(kc * M12 + m + 1) * P],
                            xTv[:, kc, tch * tch_sz:(tch + 1) * tch_sz],
                            start=(kc == 0), stop=(kc == KC - 1))
                    nc.vector.tensor_tensor(
                        out=xpv[:, m, tch * tch_sz:(tch + 1) * tch_sz]
                            .rearrange("p (s b) -> p s b", b=B),
                        in0=px[:].rearrange("p (s b) -> p s b", b=B),
                        in1=tp_b,
                        op=mybir.AluOpType.add)

            # ---- GRU recurrence + pipelined logits ----
            # State is stored SHIFTED: h_stored = h_true + 1 (so that
            # tanh(x) = 2*sigmoid(2x)-1 folds into pure mult/add DVE ops);
            # the -1 corrections are folded into biasx / bhnr / bout on the
            # host.  h_stored(t=-1) = ones.
            hring_prev = hp_pool.tile([P, 4 * KC * B], BF, tag="hring", bufs=3)
            nc.vector.memset(hring_prev[:], 1.0)
            hring = hring_prev

            ng_total = s_len // 4
            # deferred work queues: logits chunks + pass-2 (Ln+store) jobs
            logit_q = []   # (group, hring_tile, ch)
            pass2_q = []   # (group, expt_tile, rs_tile, col, ch)
            group_expt = {}
            group_sacc = {}
            pair_state = {}

            def emit_logit_chunk(job, cur_t):
                g, hr_t, ch = job
                pl = plog.tile([P, nv], F32, tag="pl", space="PSUM")
                for kc in range(KC):
                    nc.tensor.matmul(
                        pl[:], hr_t[:, kc * 4 * B:(kc + 1) * 4 * B],
                        wout_sb[:, kc * vc + ch * nv:kc * vc + (ch + 1) * nv],
                        start=(kc == 0), stop=False)
                nc.tensor.matmul(pl[:], ones_bf[:1, :P],
                                 bout_sb[:1, ch * nv:(ch + 1) * nv],
                                 start=False, stop=True)
                expt, sacc = group_expt[g], group_sacc[g]
                nc.scalar.activation(
                    expt[:, ch * nv:(ch + 1) * nv], pl[:],
                    mybir.ActivationFunctionType.Exp,
                    accum_out=sacc[:, ch:ch + 1])
                pair = g // 2
                st = pair_state[pair]
                st["done"] += 1
                if st["done"] == 2 * nch:
                    finish_pair(pair, cur_t)

            def finish_pair(pair, cur_t):
                # both groups' partial sums ready -> AllReduce -> queue pass2
                st = pair_state[pair]
                gids = st["groups"]
                ncols = len(gids)
                sred = wp.tile([P, 2], F32, tag="sred")
                for i, gg in enumerate(gids):
                    if nch > 1:
                        nc.vector.tensor_reduce(
                            sred[:, i:i + 1], group_sacc[gg][:],
                            axis=mybir.AxisListType.X, op=mybir.AluOpType.add)
                    else:
                        nc.vector.tensor_copy(sred[:, i:i + 1],
                                              group_sacc[gg][:])
                s_in = dp.tile([P, 2], F32, tag="arin")
                s_out = dp.tile([P, 2], F32, tag="arout")
                nc.gpsimd.dma_start(s_in[:, :ncols], sred[:, :ncols])
                nc.gpsimd.collective_compute(
                    "AllReduce", mybir.AluOpType.add,
                    replica_groups=[list(range(n_cores))],
                    ins=[s_in.opt()], outs=[s_out.opt()])
                sg = wp.tile([P, 2], F32, tag="sg")
                nc.gpsimd.dma_start(sg[:, :ncols], s_out[:, :ncols])
                rs = wp.tile([P, 2], F32, tag="rs", bufs=3)
                nc.vector.reciprocal(rs[:, :ncols], sg[:, :ncols])
                for i, gg in enumerate(gids):
                    for ch in range(nch):
                        # AR latency ~17us =~ 3 steps: don't let Ln ops block
                        # ScalarE's program order before the AR can finish
                        pass2_q.append((cur_t + 3,
                                        (gg, group_expt[gg], rs, i, ch)))

            def emit_pass2(job):
                gg, expt, rs, col, ch = job
                lp_t = op.tile([P, nv], F32, tag="lp")
                nc.scalar.activation(
                    lp_t[:], expt[:, ch * nv:(ch + 1) * nv],
                    mybir.ActivationFunctionType.Ln, scale=rs[:, col:col + 1])
                dst = logp_d[:, 4 * gg:4 * gg + 4, ch * nv:(ch + 1) * nv] \
                    .rearrange("b s v -> s b v")
                nc.sync.dma_start(dst, lp_t[:])

            for t in range(s_len):
                g, s_l = divmod(t, 4)
                if s_l == 0:
                    hring = hp_pool.tile([P, 4 * KC * B], BF, tag="hring",
                                         bufs=3)
                    group_expt[g] = ep.tile([P, vc], BF, tag="expg", bufs=3, name="expg")
                    group_sacc[g] = wp.tile([P, nch], F32, tag="sacc", bufs=4, name="sacc")
                    pair_state.setdefault(
                        g // 2, {"done": 0, "groups": []})["groups"].append(g)
                # ring layout: [128, kc(4) x s(4) x b(32)]
                if s_l == 0:
                    hcol = [hring_prev[:, (kc * 4 + 3) * B:(kc * 4 + 4) * B]
                            for kc in range(KC)]
                else:
                    hcol = [hring[:, (kc * 4 + s_l - 1) * B:
                                  (kc * 4 + s_l) * B] for kc in range(KC)]

                hpd = pmm.tile([P, M12 * B], F32, tag="hp", space="PSUM")
                for m in range(M12):
                    o = hpd[:, m * B:(m + 1) * B]
                    for kc in range(KC):
                        nc.tensor.matmul(
                            o,
                            whh_sb[:, (kc * M12 + m) * P:(kc * M12 + m + 1) * P],
                            hcol[kc],
                            start=(kc == 0), stop=(kc == KC - 1 and m < 2 * KC))
                    if m >= 2 * KC:
                        # fold (b_hh_n - rowsum(W_hh_n)) into the n-part
                        c = m - 2 * KC
                        nc.tensor.matmul(
                            o, bhnr_sb[:1, c * P:(c + 1) * P], ones_bf[:1, :B],
                            start=False, stop=True)

                # interleave deferred logits / pass-2 work into this step
                for _ in range(2):
                    if logit_q:
                        emit_logit_chunk(logit_q.pop(0), t)
                for _ in range(2):
                    if pass2_q and pass2_q[0][0] <= t:
                        emit_pass2(pass2_q.pop(0)[1])

                pre_rz = wp.tile([P, 2 * KC * B], F32, tag="pre_rz")
                nc.vector.tensor_tensor(
                    out=pre_rz[:].rearrange("p (m b) -> p m b", b=B),
                    in0=hpd[:, 0:2 * KC * B].rearrange("p (m b) -> p m b", b=B),
                    in1=xpv[:, 0:2 * KC, t * B:(t + 1) * B],
                    op=mybir.AluOpType.add)
                rz = wp.tile([P, 2 * KC * B], F32, tag="rz")
                nc.scalar.activation(rz[:], pre_rz[:],
                                     mybir.ActivationFunctionType.Sigmoid)

                pre_n = wp.tile([P, KC * B], F32, tag="pre_n")
                nc.vector.tensor_tensor(
                    out=pre_n[:], in0=hpd[:, 2 * KC * B:3 * KC * B],
                    in1=rz[:, 0:KC * B], op=mybir.AluOpType.mult)
                nc.vector.tensor_tensor(
                    out=pre_n[:].rearrange("p (m b) -> p m b", b=B),
                    in0=pre_n[:].rearrange("p (m b) -> p m b", b=B),
                    in1=xpv[:, 2 * KC:3 * KC, t * B:(t + 1) * B],
                    op=mybir.AluOpType.add)
                sn = wp.tile([P, KC * B], F32, tag="sn")
                nc.scalar.activation(sn[:], pre_n[:],
                                     mybir.ActivationFunctionType.Sigmoid,
                                     scale=2.0)

                # h'_stored = z*(h_stored - 2*sigma) + 2*sigma
                tt = wp.tile([P, KC * B], F32, tag="tt")
                hprev_v = (hring_prev if s_l == 0 else hring)[:] \
                    .rearrange("p (kc s b) -> p kc s b", kc=KC, s=4) \
                    [:, :, 3 if s_l == 0 else s_l - 1, :]
                nc.vector.scalar_tensor_tensor(
                    out=tt[:].rearrange("p (kc b) -> p kc b", kc=KC),
                    in0=sn[:].rearrange("p (kc b) -> p kc b", kc=KC),
                    scalar=-2.0, in1=hprev_v,
                    op0=mybir.AluOpType.mult, op1=mybir.AluOpType.add)
                uu = wp.tile([P, KC * B], F32, tag="uu")
                nc.vector.tensor_tensor(out=uu[:], in0=tt[:],
                                        in1=rz[:, KC * B:2 * KC * B],
                                        op=mybir.AluOpType.mult)
                hnew_v = hring[:].rearrange("p (kc s b) -> p kc s b",
                                            kc=KC, s=4)[:, :, s_l, :]
                nc.vector.scalar_tensor_tensor(
                    out=hnew_v,
                    in0=sn[:].rearrange("p (kc b) -> p kc b", kc=KC),
                    scalar=2.0, in1=uu[:].rearrange("p (kc b) -> p kc b", kc=KC),
                    op0=mybir.AluOpType.mult, op1=mybir.AluOpType.add)

                if s_l == 3:
                    for ch in range(nch):
                        logit_q.append((g, hring, ch))
                    hring_prev = hring

            # ---- epilogue: drain remaining logits + pass2 ----
            while logit_q:
                emit_logit_chunk(logit_q.pop(0), s_len)
            while pass2_q:
                emit_pass2(pass2_q.pop(0)[1])

            # ---- final hidden state (stored form; host subtracts 1) ----
            hfin = wp.tile([P, KC * B], F32, tag="hfin")
            nc.vector.tensor_copy(
                hfin[:].rearrange("p (kc b) -> p kc b", kc=KC),
                hring_prev[:].rearrange("p (kc s b) -> p kc s b",
                                        kc=KC, s=4)[:, :, 3, :])
            nc.sync.dma_start(
                hid_d.rearrange("(kc j) b -> j kc b", kc=KC),
                hfin[:].rearrange("p (kc b) -> p kc b", kc=KC))

    nc.compile()
    return nc


# ------------------------------------------------------------------
# host side
# ------------------------------------------------------------------

_NC_CACHE = {}


def _get_nc(n_cores, s_len, v_total):
    key = (n_cores, s_len, v_total)
    if key not in _NC_CACHE:
        _NC_CACHE[key] = build_nc(n_cores, s_len, v_total)
    return _NC_CACHE[key]


def make_in_maps(target_seqs, thought, emb, W_ih, W_hh, b_ih, b_hh, W_out,
                 b_out, n_cores=N_CORES):
    target_seqs = np.asarray(target_seqs)
    thought = np.asarray(thought, np.float32)
    emb = np.ascontiguousarray(np.asarray(emb, np.float32))
    W_ih = np.asarray(W_ih, np.float32)
    W_hh = np.asarray(W_hh, np.float32)
    b_ih = np.asarray(b_ih, np.float32)
    b_hh = np.asarray(b_hh, np.float32)
    W_out = np.asarray(W_out, np.float32)
    b_out = np.asarray(b_out, np.float32)

    b_sz, s_len = target_seqs.shape
    v_total = W_out.shape[0]
    vc = v_total // n_cores

    idx = np.ascontiguousarray(
        target_seqs.T.reshape(-1).astype(np.int32))          # tok = s*B + b
    thT = np.ascontiguousarray(thought[0].T)                 # [H, B]
    wixT = np.ascontiguousarray(W_ih[:, :H].T).astype(_BF16)
    witT = np.ascontiguousarray(W_ih[:, H:].T).astype(_BF16)
    # bf16 rounding of W_hh happens before the rowsum fold so that the
    # "-rowsum(W_hh) @ 1" correction matches the on-device bf16 matmul.
    whhT_bf = np.ascontiguousarray(W_hh.T).astype(_BF16)
    whh_rowsum = whhT_bf.astype(np.float32).sum(axis=0)      # [3H]
    # state is stored as h+1; fold -rowsum(W_hh) into the rz bias and the
    # n-gate bias row
    biasx = (b_ih + np.concatenate([b_hh[:2 * H],
                                    np.zeros(H, np.float32)])
             - np.concatenate([whh_rowsum[:2 * H], np.zeros(H, np.float32)])
             ).astype(_BF16)
    bhnr = (b_hh[2 * H:] - whh_rowsum[2 * H:]).astype(_BF16)
    woutT = np.ascontiguousarray(W_out.T.astype(_BF16))      # [H, V]
    wout_rowsum = woutT.astype(np.float32).sum(axis=0)       # [V]
    bout = (b_out - wout_rowsum).astype(_BF16)

    shared = dict(idx=idx, emb=emb, thT=thT, wixT=wixT, witT=witT,
                  whhT=whhT_bf, biasx=biasx, bhnr=bhnr)
    in_maps = []
    for c in range(n_cores):
        m = dict(shared)
        m["woutT"] = np.ascontiguousarray(woutT[:, c * vc:(c + 1) * vc])
        m["bout"] = np.ascontiguousarray(bout[c * vc:(c + 1) * vc])
        in_maps.append(m)
    return in_maps


def kernel(target_seqs, thought, emb, W_ih, W_hh, b_ih, b_hh, W_out, b_out):
    global LAST_RESULTS
    target_seqs = np.asarray(target_seqs)
    b_sz, s_len = target_seqs.shape
    v_total = np.asarray(W_out).shape[0]
    n_cores = N_CORES
    vc = v_total // n_cores

    nc = _get_nc(n_cores, s_len, v_total)
    in_maps = make_in_maps(target_seqs, thought, emb, W_ih, W_hh, b_ih, b_hh,
                           W_out, b_out, n_cores)
    res = bass_utils.run_bass_kernel_spmd(
        nc, in_maps, core_ids=list(range(n_cores)),
        trace=bool(os.environ.get("KERNEL_TRACE")))
    LAST_RESULTS = res

    logp = np.concatenate([res.results[c]["logp"] for c in range(n_cores)],
                          axis=2)
    hid = np.ascontiguousarray(res.results[0]["hid"].T - 1.0)[None]
    return logp, hid
